# revision 99
# baseline (speedup 1.0000x reference)
"""EntitySelector sparse-attention kernel for 8 Trainium2 NeuronCores.

Sharding: data-parallel over batch (16 batches -> 2 per core), no
collectives. The key restructuring vs a direct port of the reference:
the attention is over only NB=256 entities per batch, so both dense
projections are folded into entity-side precomputes instead of being
applied to the L=1024 query rows:

  scores[l,n] = q[l]·ent_k[n]          with q = query@WQ^T + bq
             = query[l]·M[:,n] + c[n]  M = A@ent^T, A = WK^T@WQ (host)
                                       c = u@ent^T, u = bq@WK  (host)
  (terms uniform over n drop out of the softmax)

  out[l,:]  = (w[l,:]/32)@ent_k@WO^T + bo
            = (w[l,:]/32)@EV + bo'    EV = ent@G, G = (WO@WK)^T (host)
                                      bo' = bo + WO@bk/32 (probs sum to 1)

so the only L-sized matmuls left are scores (L x D x NB) and the final
weighted sum (L x NB x D); everything else is NB=256 rows. Scores path
runs in fp32r (full-rate PE), value path in bf16. The output bias is a
rank-1 fp32r matmul into the same psum group; LayerNorm normalization
is a single Act op (Copy with per-partition scale/bias). DMA is
explicitly ordered: gather0 -> A -> q00 -> G -> gather1 -> q01/q10/q11,
with all batch-0 output stores deferred past the last input load.
"""

import sys

sys.path.insert(0, "/opt/trn_rl_repo")

import numpy as np
import ml_dtypes

import concourse.bass as bass
import concourse.mybir as mybir
import concourse.tile as tile
from concourse.tile_rust import add_dep_helper
from concourse import bacc
from concourse.bass_utils import run_bass_kernel_spmd
from concourse.masks import make_identity

P = 128
D = 1024
DT = D // P            # 8 feature tiles
B = 16
BL = 2                 # batches per core
L = 1024
LC = 512               # l-chunk (psum free dim)
NLC = L // LC          # 2 chunks
LT = LC // P           # 4 l-tiles per chunk
NB = 256
NT = NB // P           # 2 entity tiles
NE = 20000
NCORES = 8

F32 = mybir.dt.float32
F32R = mybir.dt.float32r
BF16 = mybir.dt.bfloat16
I32 = mybir.dt.int32

AF = mybir.ActivationFunctionType
OP = mybir.AluOpType
AX = mybir.AxisListType

_CACHE = {}


class _Ctx:
    pass


def _emit_gather(nc, g, b, idx_after=None):
    """Indirect-gather this batch's entities. idx load rides the sync
    queue; the two indirect gathers are the only SWDGE (Pool) DMAs."""
    idx_col = g.bpool.tile([P, NT], I32, tag="idxc")
    ii = nc.sync.dma_start(idx_col, g.idx[b])
    if idx_after is not None:
        add_dep_helper(ii.ins, idx_after.ins, reason="idx load ordering")

    ent_sb = []
    g_insts = []
    for nt in range(NT):
        e = g.entp.tile([P, D], F32R, tag="ent")
        gi = nc.gpsimd.indirect_dma_start(
            out=e[:], out_offset=None, in_=g.emb[:, :],
            in_offset=bass.IndirectOffsetOnAxis(ap=idx_col[:, nt:nt + 1], axis=0))
        g_insts.append(gi)
        ent_sb.append(e)
    return ent_sb, g_insts


def _emit_entT(nc, g, b, ent_sb):
    """Feature-major transposed entities, fp32r (Act copy) + bf16 (Pool)."""
    entT = g.enttp.tile([P, DT, NB], F32R, tag="entT")
    entT_bf = g.enttbp.tile([P, DT, NB], BF16, tag="entTb")
    for dt in range(DT):
        pt = g.ps_tr.tile([P, NB], F32R, tag="ptr")
        for nt in range(NT):
            nc.tensor.transpose(pt[:, nt * P:(nt + 1) * P],
                                ent_sb[nt][:, dt * P:(dt + 1) * P], g.ident_r)
        nc.scalar.mul(entT[:, dt, :], pt, 1.0)
        nc.gpsimd.tensor_copy(entT_bf[:, dt, :], entT[:, dt, :])
    return entT, entT_bf


def _emit_M(nc, g, b, entT):
    """M[di, n] = sum_e A[e, di] ent^T[e, n]  (scores rhs, fp32r)."""
    M_sb = g.mp.tile([P, DT, NB], F32R, tag="m")
    for di in range(DT):
        pm = g.ps_sc.tile([P, NB], F32, tag="psc")
        for et in range(DT):
            nc.tensor.matmul(pm, g.a_sb[:, et, di * P:(di + 1) * P],
                             entT[:, et, :],
                             start=(et == 0), stop=(et == DT - 1))
        nc.vector.tensor_copy(M_sb[:, di, :], pm)
    return M_sb


def _emit_c(nc, g, b, entT, hm_row):
    """c[n] = sum_e u[e] ent^T[e, n] + hm[n]  (scores bias row; hm is the
    -30000*(1-mask) additive mask, so masked probs underflow to 0)."""
    c_sb = g.cp.tile([1, NB], F32R, tag="c")
    pc = g.ps_sc.tile([P, NB], F32, tag="psc")
    for et in range(DT):
        nc.tensor.matmul(pc[0:1, :], g.u_sb[:, et:et + 1], entT[:, et, :],
                         start=(et == 0), stop=(et == DT - 1))
    nc.vector.tensor_add(c_sb, pc[0:1, :], hm_row)
    return c_sb


def _emit_EV(nc, g, b, entT_bf):
    """EV[n, dof] = sum_e ent[n, e] G[e, dof]  (value rows, bf16)."""
    EV = g.evp.tile([P, NT, D], BF16, tag="ev")
    for nt in range(NT):
        for half in range(2):
            pe = g.ps_big.tile([P, LC], F32, tag="pbig")
            for et in range(DT):
                nc.tensor.matmul(pe, entT_bf[:, et, nt * P:(nt + 1) * P],
                                 g.g_sb[:, et, half * LC:(half + 1) * LC],
                                 start=(et == 0), stop=(et == DT - 1))
            nc.scalar.mul(EV[:, nt, half * LC:(half + 1) * LC], pe, 1.0)
    return EV


def _emit_scores(nc, g, t, qin, M_sb, c_sb):
    """scores (additive mask via c) -> softmax -> transposed probs (bf16)."""
    psc = g.ps_sc.tile([P, NB], F32, tag="psc")
    for dt in range(DT):
        nc.tensor.matmul(psc, qin[:, dt, t * P:(t + 1) * P], M_sb[:, dt, :],
                         start=(dt == 0), stop=False)
    nc.tensor.matmul(psc, g.ones_row, c_sb, start=False, stop=True)

    negmax = g.lnp.tile([P, 1], F32, tag="nm")
    nc.vector.reduce_max(negmax, psc, axis=AX.X, negate=True)
    probs = g.probsp.tile([P, NB], BF16, tag="probs")
    rsum = g.lnp.tile([P, 1], F32, tag="rs")
    nc.scalar.activation(out=probs, in_=psc, func=AF.Exp, bias=negmax,
                         scale=1.0, accum_out=rsum)
    rinv = g.lnp.tile([P, 1], F32, tag="ri")
    nc.vector.reciprocal(rinv, rsum)
    nc.gpsimd.tensor_scalar_mul(probs, probs, rinv)
    return probs


def _emit_pT(nc, g, probs):
    """PE-transpose normalized probs to [n, l] for the final matmul."""
    ptb = g.ps_tr.tile([P, NB], BF16, tag="ptr")
    for nt in range(NT):
        nc.tensor.transpose(ptb[:, nt * P:(nt + 1) * P],
                            probs[:, nt * P:(nt + 1) * P], g.ident_b)
    pT = g.ptp.tile([P, NB], BF16, tag="pT")
    nc.vector.tensor_copy(pT, ptb)
    return pT


def _emit_out(nc, g, b, lt, pT, EV):
    """weighted EV sum (+rank-1 bias) -> LN -> store. Pool normalizes
    straight from psum; Act does only the Sqrt (Copy-compatible ops keep
    the sqrt act-table resident across the chunk's finals)."""
    po = []
    stats = g.lnp.tile([P, 2, 6], F32, tag="stats")
    for half in range(2):
        p = g.ps_big.tile([P, LC], F32, tag="pbig")
        for nt in range(NT):
            nc.tensor.matmul(p, pT[:, nt * P:(nt + 1) * P],
                             EV[:, nt, half * LC:(half + 1) * LC],
                             start=(nt == 0), stop=False)
        nc.tensor.matmul(p, g.ones_row, g.bo_row[:, half * LC:(half + 1) * LC],
                         start=False, stop=True)
        nc.vector.bn_stats(out=stats[:, half, :], in_=p)
        po.append(p)

    mv = g.lnp.tile([P, 2], F32, tag="mv")
    nc.vector.bn_aggr(out=mv, in_=stats)
    rstd = g.lnp.tile([P, 1], F32, tag="rstd")
    nc.scalar.activation(out=rstd, in_=mv[:, 1:2], func=AF.Sqrt,
                         bias=g.eps_t, scale=1.0)
    nc.vector.reciprocal(rstd, rstd)
    o_sbs = []
    for half in range(2):
        o_sb = g.opool.tile([P, LC], F32, tag=f"o{half}", name=f"o{half}")
        # plain copy frees the psum bank without waiting for the LN
        # scalars; Pool then normalizes in place (SBUF<->SBUF is legal)
        if half == 0 or g.late:
            nc.vector.tensor_copy(o_sb, po[half])
        else:
            nc.scalar.mul(o_sb, po[half], 1.0)
        o_sbs.append(o_sb)
    for half in range(2):
        o_sb = o_sbs[half]
        nc.gpsimd.tensor_scalar(out=o_sb, in0=o_sb,
                                scalar1=mv[:, 0:1], scalar2=rstd,
                                op0=OP.subtract, op1=OP.mult)
        if g.apply_affine:
            nc.vector.tensor_mul(o_sb, o_sb,
                                 g.lng_bc[:, half * LC:(half + 1) * LC])
            nc.vector.tensor_add(o_sb, o_sb,
                                 g.lnb_bc[:, half * LC:(half + 1) * LC])
        nc.scalar.dma_start(
            g.out[b, lt * P:(lt + 1) * P, half * LC:(half + 1) * LC],
            o_sb)


def _emit_scores_phase(nc, g, b, lc, M_sb, c_sb, after=None):
    """Scores + softmax + transposed probs for one 512-row chunk."""
    qTb = g.qT[b].rearrange("(kt p) l -> p kt l", p=P)
    qin = g.qinp.tile([P, DT, LC], F32R, tag="qin")
    qin_i = nc.sync.dma_start(qin, qTb[:, :, lc * LC:(lc + 1) * LC])
    if after is not None:
        add_dep_helper(qin_i.ins, after.ins, reason="qT chunk ordering")
    probses = [_emit_scores(nc, g, t, qin, M_sb, c_sb)
               for t in range(LT)]
    pTs = [_emit_pT(nc, g, pr) for pr in probses]
    return qin_i, pTs


def _emit_finals_phase(nc, g, b, lc, pTs, EV):
    for t in range(LT):
        _emit_out(nc, g, b, lc * LT + t, pTs[t], EV)
    # pre-trigger the act-table switch back to the Exp set while the Act
    # queue is idle, so the next chunk's first softmax pays no reload
    nc.scalar.activation(out=g.dummy, in_=g.dummy, func=AF.Exp)


def build_nc(apply_affine):
    nc = bacc.Bacc("TRN2", target_bir_lowering=False, debug=False,
                   num_devices=NCORES)
    g = _Ctx()
    g.apply_affine = apply_affine
    g.late = False
    g.split_ln = False

    g.qT = nc.dram_tensor("qT", [BL, D, L], F32R, kind="ExternalInput")
    g.emb = nc.dram_tensor("emb", [NE, D], F32R, kind="ExternalInput")
    g.idx = nc.dram_tensor("idx", [BL, P, NT], I32, kind="ExternalInput")
    hm_dr = nc.dram_tensor("hm", [BL, NB], F32R, kind="ExternalInput")
    a_dr = nc.dram_tensor("A", [D, D], F32R, kind="ExternalInput")
    g_dr = nc.dram_tensor("G", [D, D], BF16, kind="ExternalInput")
    u_dr = nc.dram_tensor("u", [P, DT], F32R, kind="ExternalInput")
    ones_dr = nc.dram_tensor("ones1", [P], F32R, kind="ExternalInput")
    bo_dr = nc.dram_tensor("bo", [D], F32R, kind="ExternalInput")
    if apply_affine:
        lng = nc.dram_tensor("lng", [D], F32, kind="ExternalInput")
        lnb = nc.dram_tensor("lnb", [D], F32, kind="ExternalInput")
    g.out = nc.dram_tensor("out", [BL, L, D], F32, kind="ExternalOutput")

    def as_row(dram_1d):
        ap = dram_1d[:]
        return bass.AP(tensor=ap.tensor, offset=ap.offset,
                       ap=[[0, 1]] + list(ap.ap))

    def bcast_row(dram_1d):
        ap = dram_1d[:]
        return bass.AP(tensor=ap.tensor, offset=ap.offset,
                       ap=[[0, P]] + list(ap.ap))

    with tile.TileContext(nc) as tc:
        with (
            tc.tile_pool(name="wpool", bufs=1) as wpool,
            tc.tile_pool(name="bpool", bufs=2) as bpool,
            tc.tile_pool(name="entp", bufs=2) as entp,
            tc.tile_pool(name="enttp", bufs=2) as enttp,
            tc.tile_pool(name="enttbp", bufs=2) as enttbp,
            tc.tile_pool(name="mp", bufs=2) as mp,
            tc.tile_pool(name="cp", bufs=2) as cp,
            tc.tile_pool(name="evp", bufs=2) as evp,
            tc.tile_pool(name="qinp", bufs=3) as qinp,
            tc.tile_pool(name="probsp", bufs=4) as probsp,
            tc.tile_pool(name="diagp", bufs=4) as diagp,
            tc.tile_pool(name="ptp", bufs=8) as ptp,
            tc.tile_pool(name="op", bufs=8) as opool,
            tc.tile_pool(name="lnp", bufs=4) as lnp,
            tc.tile_pool(name="ps_big", bufs=4, space="PSUM") as ps_big,
            tc.tile_pool(name="ps_sc", bufs=3, space="PSUM") as ps_sc,
            tc.tile_pool(name="ps_tr", bufs=1, space="PSUM") as ps_tr,
        ):
            g.bpool, g.entp, g.enttp, g.enttbp = bpool, entp, enttp, enttbp
            g.mp, g.cp, g.evp, g.qinp = mp, cp, evp, qinp
            g.probsp, g.ptp, g.opool, g.lnp = probsp, ptp, opool, lnp
            g.diagp = diagp
            g.ps_big, g.ps_sc, g.ps_tr = ps_big, ps_sc, ps_tr

            # DMA schedule: the shared DMA-engine resource arbitrates
            # parked requests FIFO by request time, so each load is
            # anchored (via dep) one transfer behind its intended slot --
            # the ~2us issue latency then hides under the prior transfer.
            # Target order: idx0, gather0, A, (u/bo/hm), q00, G, idx1,
            # gather1, q01, q10, q11, stores.
            ent0, g0_insts = _emit_gather(nc, g, 0)

            g.a_sb = wpool.tile([P, DT, D], F32R)
            a_r = a_dr[:, :].rearrange("(kt p) m -> p kt m", p=P)
            h = DT // 2
    # A in quarters: M's et-contraction pipelines against arrival
            a_is = []
            for qa in range(4):
                ai = nc.sync.dma_start(g.a_sb[:, 2 * qa:2 * qa + 2, :],
                                       a_r[:, 2 * qa:2 * qa + 2, :])
                add_dep_helper(ai.ins, g0_insts[0].ins,
                               reason="A anchored behind gather0a")
                a_is.append(ai)
            a_i0, a_i1 = a_is[1], a_is[3]

            g.u_sb = wpool.tile([P, DT], F32R)
            u_i = nc.sync.dma_start(g.u_sb, u_dr[:, :])
            g.bo_row = wpool.tile([1, D], F32R)
            bo_i = nc.sync.dma_start(g.bo_row, as_row(bo_dr))
            g.hm0 = wpool.tile([1, NB], F32R, name="hm0")
            hm0_i = nc.sync.dma_start(g.hm0, as_row(hm_dr[0]))
            g.hm1 = wpool.tile([1, NB], F32R, name="hm1")
            hm1_i = nc.sync.dma_start(g.hm1, as_row(hm_dr[1]))
            for si in (u_i, bo_i, hm0_i, hm1_i):
                add_dep_helper(si.ins, g0_insts[1].ins,
                               reason="small rows after gather0")
            if apply_affine:
                g.lng_bc = wpool.tile([P, D], F32)
                nc.sync.dma_start(g.lng_bc, bcast_row(lng))
                g.lnb_bc = wpool.tile([P, D], F32)
                nc.sync.dma_start(g.lnb_bc, bcast_row(lnb))

            # setup constants (after DMA issue: keeps engine queues clear)
            ident = wpool.tile([P, P], F32)
            make_identity(nc, ident)
            g.ident_r = wpool.tile([P, P], F32R)
            nc.vector.tensor_copy(g.ident_r, ident)
            g.ident_b = wpool.tile([P, P], BF16)
            nc.vector.tensor_copy(g.ident_b, ident)
            g.eps_t = wpool.tile([P, 1], F32)
            nc.vector.memset(g.eps_t, 1e-5)
            g.neg1 = wpool.tile([P, 1], F32)
            nc.vector.memset(g.neg1, -1.0)
            g.dummy = wpool.tile([1, 1], F32)
            nc.vector.memset(g.dummy, 1.0)
            g.ones_row = wpool.tile([1, P], F32R)
            nc.gpsimd.dma_start(g.ones_row, as_row(ones_dr))

            # batch 0 prep
            entT0, entT0_bf = _emit_entT(nc, g, 0, ent0)
            M0 = _emit_M(nc, g, 0, entT0)
            c0 = _emit_c(nc, g, 0, entT0, g.hm0)

            g.g_sb = wpool.tile([P, DT, D], BF16)
            g_r = g_dr[:, :].rearrange("(kt p) m -> p kt m", p=P)

            # batch 1 gathers: emitted BEFORE chunk b0c0 so their Pool-queue
            # descriptor generation isn't stuck behind the chunk's Pool ops;
            # parks early enough to win the resource right after G.
            ent1, g1_insts = _emit_gather(nc, g, 1, idx_after=g0_insts[0])
            for gi in g1_insts:
                add_dep_helper(gi.ins, a_i0.ins,
                               reason="gather1 anchored behind A0")

            # software pipeline across the four chunks: each chunk's finals
            # are emitted after the NEXT chunk's scores, so the softmax
            # round-trip latency hides under other PE work.
            q00_i, pT00 = _emit_scores_phase(nc, g, 0, 0, M0, c0,
                                             after=g0_insts[1])
            for qg in range(2):
                g_i = nc.sync.dma_start(g.g_sb[:, 4 * qg:4 * qg + 4, :],
                                        g_r[:, 4 * qg:4 * qg + 4, :])
                add_dep_helper(g_i.ins, a_i0.ins,
                               reason="G anchored behind A0 (runs after q00)")
            EV0 = _emit_EV(nc, g, 0, entT0_bf)

            q01_i, pT01 = _emit_scores_phase(nc, g, 0, 1, M0, c0,
                                             after=q00_i)
            _emit_finals_phase(nc, g, 0, 0, pT00, EV0)

            # batch 1 prep
            entT1, entT1_bf = _emit_entT(nc, g, 1, ent1)
            M1 = _emit_M(nc, g, 1, entT1)
            c1 = _emit_c(nc, g, 1, entT1, g.hm1)

            q10_i, pT10 = _emit_scores_phase(nc, g, 1, 0, M1, c1,
                                             after=q00_i)
            _emit_finals_phase(nc, g, 0, 1, pT01, EV0)

            EV1 = _emit_EV(nc, g, 1, entT1_bf)

            q11_i, pT11 = _emit_scores_phase(nc, g, 1, 1, M1, c1,
                                             after=q01_i)
            # tail: interleave the last two finals phases per tile so the
            # trailing LN/store chains overlap
            for t in range(LT):
                g.late = (t == LT - 1)
                _emit_out(nc, g, 1, t, pT10[t], EV1)
                _emit_out(nc, g, 1, LT + t, pT11[t], EV1)

    nc.compile()
    return nc


def _get_nc(apply_affine):
    key = bool(apply_affine)
    if key not in _CACHE:
        _CACHE[key] = build_nc(key)
    return _CACHE[key]


def kernel(query, ent_emb, ent_idx_in_batch, max_entity_number,
           WQ_w, WQ_b, WK_w, WK_b, WO_w, WO_b, ln_g, ln_b):
    query = np.asarray(query, np.float32)
    ent_emb = np.ascontiguousarray(np.asarray(ent_emb, np.float32))
    idx = np.asarray(ent_idx_in_batch)
    mask = (idx != -1).astype(np.float32)
    hm = np.ascontiguousarray(-30000.0 * (1.0 - mask)).astype(np.float32)
    idx32 = np.where(idx < 0, 0, idx).astype(np.int32)
    idxT = np.ascontiguousarray(
        idx32.reshape(B, NT, P).transpose(0, 2, 1))  # [B, P, NT]

    wq = np.asarray(WQ_w, np.float64)
    wk = np.asarray(WK_w, np.float64)
    wo = np.asarray(WO_w, np.float64)
    bq = np.asarray(WQ_b, np.float64)
    bk = np.asarray(WK_b, np.float64)
    bo = np.asarray(WO_b, np.float64)

    A = np.ascontiguousarray((wk.T @ wq).astype(np.float32))          # [e, di]
    G = np.ascontiguousarray(
        ((wo @ wk).T * (float(D) ** -0.5)).astype(ml_dtypes.bfloat16))
    u = np.ascontiguousarray(
        (bq @ wk).astype(np.float32).reshape(DT, P).T)                # [P, DT]
    bo_eff = np.ascontiguousarray(
        (bo + (wo @ bk) * (float(D) ** -0.5)).astype(np.float32))

    lng = np.asarray(ln_g, np.float32)
    lnb = np.asarray(ln_b, np.float32)
    apply_affine = not (np.all(lng == 1.0) and np.all(lnb == 0.0))

    qT = np.ascontiguousarray(query.transpose(0, 2, 1))  # (B, D, L)

    nc = _get_nc(apply_affine)
    in_maps = []
    for c in range(NCORES):
        s = slice(c * BL, (c + 1) * BL)
        m = dict(
            qT=np.ascontiguousarray(qT[s]),
            emb=ent_emb,
            idx=np.ascontiguousarray(idxT[s]),
            hm=np.ascontiguousarray(hm[s]),
            A=A, G=G, u=u, bo=bo_eff, ones1=np.ones(P, np.float32),
        )
        if apply_affine:
            m["lng"] = lng
            m["lnb"] = lnb
        in_maps.append(m)

    res = run_bass_kernel_spmd(nc, in_maps, core_ids=list(range(NCORES)))
    return np.concatenate([r["out"] for r in res.results], axis=0)


# revision 103
# speedup vs baseline: 1.0383x; 1.0383x over previous
"""EntitySelector sparse-attention kernel for 8 Trainium2 NeuronCores.

Sharding: data-parallel over batch (16 batches -> 2 per core), no
collectives. The key restructuring vs a direct port of the reference:
the attention is over only NB=256 entities per batch, so both dense
projections are folded into entity-side precomputes instead of being
applied to the L=1024 query rows:

  scores[l,n] = q[l]·ent_k[n]          with q = query@WQ^T + bq
             = query[l]·M[:,n] + c[n]  M = A@ent^T, A = WK^T@WQ (host)
                                       c = u@ent^T, u = bq@WK  (host)
  (terms uniform over n drop out of the softmax)

  out[l,:]  = (w[l,:]/32)@ent_k@WO^T + bo
            = (w[l,:]/32)@EV + bo'    EV = ent@G, G = (WO@WK)^T (host)
                                      bo' = bo + WO@bk/32 (probs sum to 1)

so the only L-sized matmuls left are scores (L x D x NB) and the final
weighted sum (L x NB x D); everything else is NB=256 rows. Scores path
runs in fp32r (full-rate PE), value path in bf16. The output bias is a
rank-1 fp32r matmul into the same psum group; LayerNorm normalization
is a single Act op (Copy with per-partition scale/bias). DMA is
explicitly ordered: gather0 -> A -> q00 -> G -> gather1 -> q01/q10/q11,
with all batch-0 output stores deferred past the last input load.
"""

import sys

sys.path.insert(0, "/opt/trn_rl_repo")

import numpy as np
import ml_dtypes

import concourse.bass as bass
import concourse.mybir as mybir
import concourse.tile as tile
from concourse.tile_rust import add_dep_helper
from concourse import bacc
from concourse.bass_utils import run_bass_kernel_spmd
from concourse.masks import make_identity

P = 128
D = 1024
DT = D // P            # 8 feature tiles
B = 16
BL = 2                 # batches per core
L = 1024
LC = 512               # l-chunk (psum free dim)
NLC = L // LC          # 2 chunks
LT = LC // P           # 4 l-tiles per chunk
NB = 256
NT = NB // P           # 2 entity tiles
NE = 20000
NCORES = 8

F32 = mybir.dt.float32
F32R = mybir.dt.float32r
BF16 = mybir.dt.bfloat16
I32 = mybir.dt.int32

AF = mybir.ActivationFunctionType
OP = mybir.AluOpType
AX = mybir.AxisListType

_CACHE = {}


class _Ctx:
    pass


def _emit_gather(nc, g, b, idx_after=None):
    """Indirect-gather this batch's entities. idx load rides the sync
    queue; the two indirect gathers are the only SWDGE (Pool) DMAs."""
    idx_col = g.bpool.tile([P, NT], I32, tag="idxc")
    ii = nc.sync.dma_start(idx_col, g.idx[b])
    if idx_after is not None:
        add_dep_helper(ii.ins, idx_after.ins, reason="idx load ordering")

    ent_sb = []
    g_insts = []
    for nt in range(NT):
        e = g.entp.tile([P, D], F32R, tag="ent")
        gi = nc.gpsimd.indirect_dma_start(
            out=e[:], out_offset=None, in_=g.emb[:, :],
            in_offset=bass.IndirectOffsetOnAxis(ap=idx_col[:, nt:nt + 1], axis=0))
        g_insts.append(gi)
        ent_sb.append(e)
    return ent_sb, g_insts


def _emit_entT(nc, g, b, ent_sb):
    """Feature-major transposed entities, fp32r (Act copy) + bf16 (Pool)."""
    entT = g.enttp.tile([P, DT, NB], F32R, tag="entT")
    entT_bf = g.enttbp.tile([P, DT, NB], BF16, tag="entTb")
    for dt in range(DT):
        pt = g.ps_tr.tile([P, NB], F32R, tag="ptr")
        for nt in range(NT):
            nc.tensor.transpose(pt[:, nt * P:(nt + 1) * P],
                                ent_sb[nt][:, dt * P:(dt + 1) * P], g.ident_r)
        nc.scalar.mul(entT[:, dt, :], pt, 1.0)
        nc.gpsimd.tensor_copy(entT_bf[:, dt, :], entT[:, dt, :])
    return entT, entT_bf


def _emit_M(nc, g, b, entT):
    """M[di, n] = sum_e A[e, di] ent^T[e, n]  (scores rhs, fp32r)."""
    M_sb = g.mp.tile([P, DT, NB], F32R, tag="m")
    for di in range(DT):
        pm = g.ps_sc.tile([P, NB], F32, tag="psc")
        for et in range(DT):
            nc.tensor.matmul(pm, g.a_sb[:, et, di * P:(di + 1) * P],
                             entT[:, et, :],
                             start=(et == 0), stop=(et == DT - 1))
        nc.vector.tensor_copy(M_sb[:, di, :], pm)
    return M_sb


def _emit_c(nc, g, b, entT, hm_row):
    """c[n] = sum_e u[e] ent^T[e, n] + hm[n]  (scores bias row; hm is the
    -30000*(1-mask) additive mask, so masked probs underflow to 0)."""
    c_sb = g.cp.tile([1, NB], F32R, tag="c")
    pc = g.ps_sc.tile([P, NB], F32, tag="psc")
    for et in range(DT):
        nc.tensor.matmul(pc[0:1, :], g.u_sb[:, et:et + 1], entT[:, et, :],
                         start=(et == 0), stop=(et == DT - 1))
    nc.vector.tensor_add(c_sb, pc[0:1, :], hm_row)
    return c_sb


def _emit_EV(nc, g, b, entT_bf):
    """EV[n, dof] = sum_e ent[n, e] G[e, dof]  (value rows, bf16)."""
    EV = g.evp.tile([P, NT, D], BF16, tag="ev")
    for nt in range(NT):
        for half in range(2):
            pe = g.ps_big.tile([P, LC], F32, tag="pbig")
            for et in range(DT):
                nc.tensor.matmul(pe, entT_bf[:, et, nt * P:(nt + 1) * P],
                                 g.g_sb[:, et, half * LC:(half + 1) * LC],
                                 start=(et == 0), stop=(et == DT - 1))
            nc.scalar.mul(EV[:, nt, half * LC:(half + 1) * LC], pe, 1.0)
    return EV


def _emit_scores(nc, g, t, qin, M_sb, c_sb):
    """scores (additive mask via c) -> softmax -> transposed probs (bf16)."""
    psc = g.ps_sc.tile([P, NB], F32, tag="psc")
    for dt in range(DT):
        nc.tensor.matmul(psc, qin[:, dt, t * P:(t + 1) * P], M_sb[:, dt, :],
                         start=(dt == 0), stop=False)
    nc.tensor.matmul(psc, g.ones_row, c_sb, start=False, stop=True)

    negmax = g.lnp.tile([P, 1], F32, tag="nm")
    nc.vector.reduce_max(negmax, psc, axis=AX.X, negate=True)
    probs = g.probsp.tile([P, NB], BF16, tag="probs")
    rsum = g.lnp.tile([P, 1], F32, tag="rs")
    nc.scalar.activation(out=probs, in_=psc, func=AF.Exp, bias=negmax,
                         scale=1.0, accum_out=rsum)
    rinv = g.lnp.tile([P, 1], F32, tag="ri")
    nc.vector.reciprocal(rinv, rsum)
    nc.gpsimd.tensor_scalar_mul(probs, probs, rinv)
    return probs


def _emit_pT(nc, g, probs):
    """PE-transpose normalized probs to [n, l] for the final matmul."""
    ptb = g.ps_tr.tile([P, NB], BF16, tag="ptr")
    for nt in range(NT):
        nc.tensor.transpose(ptb[:, nt * P:(nt + 1) * P],
                            probs[:, nt * P:(nt + 1) * P], g.ident_b)
    pT = g.ptp.tile([P, NB], BF16, tag="pT")
    nc.vector.tensor_copy(pT, ptb)
    return pT


def _emit_out(nc, g, b, lt, pT, EV):
    """weighted EV sum (+rank-1 bias) -> LN -> store. Pool normalizes
    straight from psum; Act does only the Sqrt (Copy-compatible ops keep
    the sqrt act-table resident across the chunk's finals)."""
    po = []
    stats = g.lnp.tile([P, 2, 6], F32, tag="stats")
    for half in range(2):
        p = g.ps_big.tile([P, LC], F32, tag="pbig")
        for nt in range(NT):
            nc.tensor.matmul(p, pT[:, nt * P:(nt + 1) * P],
                             EV[:, nt, half * LC:(half + 1) * LC],
                             start=(nt == 0), stop=False)
        nc.tensor.matmul(p, g.ones_row, g.bo_row[:, half * LC:(half + 1) * LC],
                         start=False, stop=True)
        nc.vector.bn_stats(out=stats[:, half, :], in_=p)
        po.append(p)

    mv = g.lnp.tile([P, 2], F32, tag="mv")
    nc.vector.bn_aggr(out=mv, in_=stats)
    rstd = g.lnp.tile([P, 1], F32, tag="rstd")
    nc.scalar.activation(out=rstd, in_=mv[:, 1:2], func=AF.Sqrt,
                         bias=g.eps_t, scale=1.0)
    nc.vector.reciprocal(rstd, rstd)
    o_sbs = []
    for half in range(2):
        o_sb = g.opool.tile([P, LC], F32, tag=f"o{half}", name=f"o{half}")
        # plain copy frees the psum bank without waiting for the LN
        # scalars; Pool then normalizes in place (SBUF<->SBUF is legal)
        if half == 0 or g.late:
            nc.vector.tensor_copy(o_sb, po[half])
        else:
            nc.scalar.mul(o_sb, po[half], 1.0)
        o_sbs.append(o_sb)
    for half in range(2):
        o_sb = o_sbs[half]
        nc.gpsimd.tensor_scalar(out=o_sb, in0=o_sb,
                                scalar1=mv[:, 0:1], scalar2=rstd,
                                op0=OP.subtract, op1=OP.mult)
        if g.apply_affine:
            nc.vector.tensor_mul(o_sb, o_sb,
                                 g.lng_bc[:, half * LC:(half + 1) * LC])
            nc.vector.tensor_add(o_sb, o_sb,
                                 g.lnb_bc[:, half * LC:(half + 1) * LC])
        nc.sync.dma_start(
            g.out[b, lt * P:(lt + 1) * P, half * LC:(half + 1) * LC],
            o_sb)


def _emit_scores_phase(nc, g, b, lc, M_sb, c_sb, after=None):
    """Scores + softmax + transposed probs for one 512-row chunk."""
    qTb = g.qT[b].rearrange("(kt p) l -> p kt l", p=P)
    qin = g.qinp.tile([P, DT, LC], F32R, tag="qin")
    qin_i = nc.sync.dma_start(qin, qTb[:, :, lc * LC:(lc + 1) * LC])
    if after is not None:
        add_dep_helper(qin_i.ins, after.ins, reason="qT chunk ordering")
    probses = [_emit_scores(nc, g, t, qin, M_sb, c_sb)
               for t in range(LT)]
    pTs = [_emit_pT(nc, g, pr) for pr in probses]
    return qin_i, pTs


def _emit_finals_phase(nc, g, b, lc, pTs, EV):
    for t in range(LT):
        _emit_out(nc, g, b, lc * LT + t, pTs[t], EV)
    # pre-trigger the act-table switch back to the Exp set while the Act
    # queue is idle, so the next chunk's first softmax pays no reload
    nc.scalar.activation(out=g.dummy, in_=g.dummy, func=AF.Exp)


def build_nc(apply_affine):
    nc = bacc.Bacc("TRN2", target_bir_lowering=False, debug=False,
                   num_devices=NCORES)
    g = _Ctx()
    g.apply_affine = apply_affine
    g.late = False
    g.split_ln = False

    g.qT = nc.dram_tensor("qT", [BL, D, L], F32R, kind="ExternalInput")
    g.emb = nc.dram_tensor("emb", [NE, D], F32R, kind="ExternalInput")
    g.idx = nc.dram_tensor("idx", [BL, P, NT], I32, kind="ExternalInput")
    hm_dr = nc.dram_tensor("hm", [BL, NB], F32R, kind="ExternalInput")
    a_dr = nc.dram_tensor("A", [D, D], F32R, kind="ExternalInput")
    g_dr = nc.dram_tensor("G", [D, D], BF16, kind="ExternalInput")
    u_dr = nc.dram_tensor("u", [P, DT], F32R, kind="ExternalInput")
    ones_dr = nc.dram_tensor("ones1", [P], F32R, kind="ExternalInput")
    bo_dr = nc.dram_tensor("bo", [D], F32R, kind="ExternalInput")
    if apply_affine:
        lng = nc.dram_tensor("lng", [D], F32, kind="ExternalInput")
        lnb = nc.dram_tensor("lnb", [D], F32, kind="ExternalInput")
    g.out = nc.dram_tensor("out", [BL, L, D], F32, kind="ExternalOutput")

    def as_row(dram_1d):
        ap = dram_1d[:]
        return bass.AP(tensor=ap.tensor, offset=ap.offset,
                       ap=[[0, 1]] + list(ap.ap))

    def bcast_row(dram_1d):
        ap = dram_1d[:]
        return bass.AP(tensor=ap.tensor, offset=ap.offset,
                       ap=[[0, P]] + list(ap.ap))

    with tile.TileContext(nc) as tc:
        with (
            tc.tile_pool(name="wpool", bufs=1) as wpool,
            tc.tile_pool(name="bpool", bufs=2) as bpool,
            tc.tile_pool(name="entp", bufs=2) as entp,
            tc.tile_pool(name="enttp", bufs=2) as enttp,
            tc.tile_pool(name="enttbp", bufs=2) as enttbp,
            tc.tile_pool(name="mp", bufs=2) as mp,
            tc.tile_pool(name="cp", bufs=2) as cp,
            tc.tile_pool(name="evp", bufs=2) as evp,
            tc.tile_pool(name="qinp", bufs=3) as qinp,
            tc.tile_pool(name="probsp", bufs=4) as probsp,
            tc.tile_pool(name="diagp", bufs=4) as diagp,
            tc.tile_pool(name="ptp", bufs=8) as ptp,
            tc.tile_pool(name="op", bufs=8) as opool,
            tc.tile_pool(name="lnp", bufs=4) as lnp,
            tc.tile_pool(name="ps_big", bufs=4, space="PSUM") as ps_big,
            tc.tile_pool(name="ps_sc", bufs=3, space="PSUM") as ps_sc,
            tc.tile_pool(name="ps_tr", bufs=1, space="PSUM") as ps_tr,
        ):
            g.bpool, g.entp, g.enttp, g.enttbp = bpool, entp, enttp, enttbp
            g.mp, g.cp, g.evp, g.qinp = mp, cp, evp, qinp
            g.probsp, g.ptp, g.opool, g.lnp = probsp, ptp, opool, lnp
            g.diagp = diagp
            g.ps_big, g.ps_sc, g.ps_tr = ps_big, ps_sc, ps_tr

            # DMA schedule: the shared DMA-engine resource arbitrates
            # parked requests FIFO by request time, so each load is
            # anchored (via dep) one transfer behind its intended slot --
            # the ~2us issue latency then hides under the prior transfer.
            # Target order: idx0, gather0, A, (u/bo/hm), q00, G, idx1,
            # gather1, q01, q10, q11, stores.
            ent0, g0_insts = _emit_gather(nc, g, 0)

            g.a_sb = wpool.tile([P, DT, D], F32R)
            a_r = a_dr[:, :].rearrange("(kt p) m -> p kt m", p=P)
            h = DT // 2
    # A in quarters: M's et-contraction pipelines against arrival
            a_is = []
            for qa in range(4):
                ai = nc.sync.dma_start(g.a_sb[:, 2 * qa:2 * qa + 2, :],
                                       a_r[:, 2 * qa:2 * qa + 2, :])
                add_dep_helper(ai.ins, g0_insts[0].ins,
                               reason="A anchored behind gather0a")
                a_is.append(ai)
            a_i0, a_i1 = a_is[1], a_is[3]

            g.u_sb = wpool.tile([P, DT], F32R)
            u_i = nc.sync.dma_start(g.u_sb, u_dr[:, :])
            g.bo_row = wpool.tile([1, D], F32R)
            bo_i = nc.sync.dma_start(g.bo_row, as_row(bo_dr))
            g.hm0 = wpool.tile([1, NB], F32R, name="hm0")
            hm0_i = nc.sync.dma_start(g.hm0, as_row(hm_dr[0]))
            g.hm1 = wpool.tile([1, NB], F32R, name="hm1")
            hm1_i = nc.sync.dma_start(g.hm1, as_row(hm_dr[1]))
            for si in (u_i, bo_i, hm0_i, hm1_i):
                add_dep_helper(si.ins, g0_insts[1].ins,
                               reason="small rows after gather0")
            if apply_affine:
                g.lng_bc = wpool.tile([P, D], F32)
                nc.sync.dma_start(g.lng_bc, bcast_row(lng))
                g.lnb_bc = wpool.tile([P, D], F32)
                nc.sync.dma_start(g.lnb_bc, bcast_row(lnb))

            # setup constants (after DMA issue: keeps engine queues clear)
            ident = wpool.tile([P, P], F32)
            make_identity(nc, ident)
            g.ident_r = wpool.tile([P, P], F32R)
            nc.vector.tensor_copy(g.ident_r, ident)
            g.ident_b = wpool.tile([P, P], BF16)
            nc.vector.tensor_copy(g.ident_b, ident)
            g.eps_t = wpool.tile([P, 1], F32)
            nc.vector.memset(g.eps_t, 1e-5)
            g.neg1 = wpool.tile([P, 1], F32)
            nc.vector.memset(g.neg1, -1.0)
            g.dummy = wpool.tile([1, 1], F32)
            nc.vector.memset(g.dummy, 1.0)
            g.ones_row = wpool.tile([1, P], F32R)
            nc.gpsimd.dma_start(g.ones_row, as_row(ones_dr))

            # batch 0 prep
            entT0, entT0_bf = _emit_entT(nc, g, 0, ent0)
            M0 = _emit_M(nc, g, 0, entT0)
            c0 = _emit_c(nc, g, 0, entT0, g.hm0)

            g.g_sb = wpool.tile([P, DT, D], BF16)
            g_r = g_dr[:, :].rearrange("(kt p) m -> p kt m", p=P)

            # batch 1 gathers: emitted BEFORE chunk b0c0 so their Pool-queue
            # descriptor generation isn't stuck behind the chunk's Pool ops;
            # parks early enough to win the resource right after G.
            ent1, g1_insts = _emit_gather(nc, g, 1, idx_after=g0_insts[0])
            for gi in g1_insts:
                add_dep_helper(gi.ins, a_i0.ins,
                               reason="gather1 anchored behind A0")

            # software pipeline across the four chunks: each chunk's finals
            # are emitted after the NEXT chunk's scores, so the softmax
            # round-trip latency hides under other PE work.
            q00_i, pT00 = _emit_scores_phase(nc, g, 0, 0, M0, c0,
                                             after=g0_insts[1])
            for qg in range(2):
                g_i = nc.sync.dma_start(g.g_sb[:, 4 * qg:4 * qg + 4, :],
                                        g_r[:, 4 * qg:4 * qg + 4, :])
                add_dep_helper(g_i.ins, a_i0.ins,
                               reason="G anchored behind A0 (runs after q00)")
            EV0 = _emit_EV(nc, g, 0, entT0_bf)

            q01_i, pT01 = _emit_scores_phase(nc, g, 0, 1, M0, c0,
                                             after=q00_i)
            _emit_finals_phase(nc, g, 0, 0, pT00, EV0)

            # batch 1 prep
            entT1, entT1_bf = _emit_entT(nc, g, 1, ent1)
            M1 = _emit_M(nc, g, 1, entT1)
            c1 = _emit_c(nc, g, 1, entT1, g.hm1)

            q10_i, pT10 = _emit_scores_phase(nc, g, 1, 0, M1, c1,
                                             after=q00_i)
            _emit_finals_phase(nc, g, 0, 1, pT01, EV0)

            EV1 = _emit_EV(nc, g, 1, entT1_bf)

            q11_i, pT11 = _emit_scores_phase(nc, g, 1, 1, M1, c1,
                                             after=q01_i)
            # tail: interleave the last two finals phases per tile so the
            # trailing LN/store chains overlap
            for t in range(LT):
                g.late = (t == LT - 1)
                _emit_out(nc, g, 1, t, pT10[t], EV1)
                _emit_out(nc, g, 1, LT + t, pT11[t], EV1)

    nc.compile()
    return nc


def _get_nc(apply_affine):
    key = bool(apply_affine)
    if key not in _CACHE:
        _CACHE[key] = build_nc(key)
    return _CACHE[key]


def kernel(query, ent_emb, ent_idx_in_batch, max_entity_number,
           WQ_w, WQ_b, WK_w, WK_b, WO_w, WO_b, ln_g, ln_b):
    query = np.asarray(query, np.float32)
    ent_emb = np.ascontiguousarray(np.asarray(ent_emb, np.float32))
    idx = np.asarray(ent_idx_in_batch)
    mask = (idx != -1).astype(np.float32)
    hm = np.ascontiguousarray(-30000.0 * (1.0 - mask)).astype(np.float32)
    idx32 = np.where(idx < 0, 0, idx).astype(np.int32)
    idxT = np.ascontiguousarray(
        idx32.reshape(B, NT, P).transpose(0, 2, 1))  # [B, P, NT]

    wq = np.asarray(WQ_w, np.float64)
    wk = np.asarray(WK_w, np.float64)
    wo = np.asarray(WO_w, np.float64)
    bq = np.asarray(WQ_b, np.float64)
    bk = np.asarray(WK_b, np.float64)
    bo = np.asarray(WO_b, np.float64)

    A = np.ascontiguousarray((wk.T @ wq).astype(np.float32))          # [e, di]
    G = np.ascontiguousarray(
        ((wo @ wk).T * (float(D) ** -0.5)).astype(ml_dtypes.bfloat16))
    u = np.ascontiguousarray(
        (bq @ wk).astype(np.float32).reshape(DT, P).T)                # [P, DT]
    bo_eff = np.ascontiguousarray(
        (bo + (wo @ bk) * (float(D) ** -0.5)).astype(np.float32))

    lng = np.asarray(ln_g, np.float32)
    lnb = np.asarray(ln_b, np.float32)
    apply_affine = not (np.all(lng == 1.0) and np.all(lnb == 0.0))

    qT = np.ascontiguousarray(query.transpose(0, 2, 1))  # (B, D, L)

    nc = _get_nc(apply_affine)
    in_maps = []
    for c in range(NCORES):
        s = slice(c * BL, (c + 1) * BL)
        m = dict(
            qT=np.ascontiguousarray(qT[s]),
            emb=ent_emb,
            idx=np.ascontiguousarray(idxT[s]),
            hm=np.ascontiguousarray(hm[s]),
            A=A, G=G, u=u, bo=bo_eff, ones1=np.ones(P, np.float32),
        )
        if apply_affine:
            m["lng"] = lng
            m["lnb"] = lnb
        in_maps.append(m)

    res = run_bass_kernel_spmd(nc, in_maps, core_ids=list(range(NCORES)))
    return np.concatenate([r["out"] for r in res.results], axis=0)
